# revision 22
# baseline (speedup 1.0000x reference)
"""Trainium2 Bass kernel for nn_EyeRobotAgent block-sparse ("eye") attention.

Shapes: q,k,v [2, 12, 3456, 32] fp32.  S = 16 time-blocks x 216 feats.
Visibility per query block t:
  - g1 keys (visible to ALL 216 q of the block): the 20 non-img keys of
    block t, plus 19 keys (m in {0..3, 5..19}) of each past block
    t-7..t-1 (joint q additionally can't see past joint keys -> folded
    into the QK matmul as a bias row),
  - g2 keys (visible ONLY to the 20 non-img q): the 196 img keys of
    block t (img q never see img keys -> bias row for mixed chunks).

Strategy (data-parallel: 24 (b,h) pairs over 8 cores, 3 each):
  Per block, pack [g1 | g2] key rows into 128-partition chunks; chunks
  containing any g1 row stream all 216 q columns, pure-g2 chunks stream
  only the 20 non-img q columns.  Early blocks have fewer past keys and
  fewer/smaller chunks.  Scores are computed transposed [kv, q] with the
  two mask predicates folded in as 2 extra contraction rows; exp runs
  split between the Act engine (Exp) and the DVE (pow(e, x)) so neither
  is a serial bottleneck; probs feed PV matmuls oriented out[q, 33]
  (33-column streams; column 32 is the softmax denominator via a ones
  column in V).  PV accumulates 8 blocks (a half-(b,h)) per PSUM bank,
  then one batched reciprocal+multiply normalizes and stores.
"""
import numpy as np

import concourse.bass as bass
import concourse.mybir as mybir
import concourse.tile as tile
from concourse import bacc
from concourse.bass_utils import run_bass_kernel_spmd
from concourse.tile_rust import add_dep_helper

B, H, S, D = 2, 12, 3456, 32
F = 216            # feats_per_t
W = 8              # window_len
T = S // F         # 16 blocks
IMG_START = 20     # F - img_feat_size
JOINT_START = 4    # IMG_START - act_size
PAST_SEL = np.array([0, 1, 2, 3] + list(range(5, 20)))   # 19 per past block
NEG = np.float32(-30000.0)
SCALE = float(1.0 / np.sqrt(np.float32(D)))
N_CORES = 8
BH_PER_CORE = (B * H) // N_CORES      # 3
KAUG = D + 2       # 32 d + img-bias row + joint-bias row
VA = D + 1         # 33 = v columns + ones column

F32 = mybir.dt.float32
F16 = mybir.dt.float16
NP_F16 = np.float16


# ------------------------------------------------------------- chunk schedule
def _schedule():
    """Per block t: list of chunks dict(rows, cols, keys, koff, ci)."""
    scheds = []
    koff = 0
    ci = 0
    for t in range(T):
        g1 = [t * F + m for m in range(IMG_START)]
        for tau in range(max(0, t - (W - 1)), t):
            g1 += [tau * F + int(m) for m in PAST_SEL]
        g2 = [t * F + m for m in range(IMG_START, F)]
        keys = np.array(g1 + g2)
        n1 = len(g1)
        chunks = []
        off = 0
        while off < len(keys):
            n = int(min(128, len(keys) - off))
            cols = F if off < n1 else IMG_START
            chunks.append(dict(rows=n, cols=cols, keys=keys[off:off + n],
                               koff=koff, ci=ci))
            koff += n
            ci += 1
            off += n
        scheds.append(chunks)
    return scheds, koff, ci


SCHED, TOT, NCH = _schedule()
BLK_COLS = [sum(c["cols"] for c in SCHED[t]) for t in range(T)]
PAIR_COLS = [max(BLK_COLS[2 * p], BLK_COLS[2 * p + 1]) for p in range(T // 2)]
KHALF = sum(c["rows"] for t in range(T // 2) for c in SCHED[t])
CHALF = sum(len(SCHED[t]) for t in range(T // 2))

# exp engine split per pair: Act handles cols [0, a), DVE handles [a, cmax)
# (DVE is ~1.25x slower per col and also runs the reciprocals; Act pays a
# bigger fixed per-op cost).  Tuned against the CoreSim cost model.
ACT_COLS = [170, 184, 184, 325, 325, 325, 325, 325]

NORM_ENGINE = "vector"     # engine for the normalize multiplies
DMA_ENGINE = "sync"        # SP-issued DMAs run async on the HWDGE queues


# ---------------------------------------------------------------- host packing
def _pack_all(q, k, v):
    """q,k,v: [B,H,S,D] fp32 ->
       qt  [24, KAUG, S]    (augmented scaled Q^T)
       kpt [24, KAUG, TOT]  (augmented packed K^T, chunk-major)
       vp  [24, 128, NCH*VA] (packed V + ones column per chunk)"""
    nbh = B * H
    qf = q.reshape(nbh, S, D)
    kf = k.reshape(nbh, S, D)
    vf = v.reshape(nbh, S, D)

    qm = np.arange(S) % F
    qt = np.zeros((nbh, KAUG, S), np.float32)
    qt[:, :D] = (qf * SCALE).transpose(0, 2, 1)
    qt[:, 32] = qm >= IMG_START
    qt[:, 33] = (qm >= JOINT_START) & (qm < IMG_START)

    keys_all = np.concatenate([c["keys"] for t in range(T) for c in SCHED[t]])
    km = keys_all % F
    kt = keys_all // F
    # block id each key row belongs to (for past-joint bias)
    owner = np.concatenate([np.full(c["rows"], t)
                            for t in range(T) for c in SCHED[t]])
    kpt = np.zeros((nbh, KAUG, TOT + 128), np.float32)
    kpt[:, :D, :TOT] = kf[:, keys_all].transpose(0, 2, 1)
    kpt[:, 32, :TOT] = np.where(km >= IMG_START, NEG, 0.0)
    kpt[:, 33, :TOT] = np.where((km >= JOINT_START) & (km < IMG_START)
                                & (kt < owner), NEG, 0.0)

    vp = np.zeros((nbh, NCH, 128, VA), np.float32)
    for t in range(T):
        for c in SCHED[t]:
            r = c["rows"]
            vp[:, c["ci"], :r, :D] = vf[:, c["keys"]]
            vp[:, c["ci"], :r, 32] = 1.0
    vp = np.ascontiguousarray(
        vp.transpose(0, 2, 1, 3).reshape(nbh, 128, NCH * VA))
    return (qt.astype(NP_F16), kpt.astype(NP_F16), vp.astype(NP_F16))


# ---------------------------------------------------------------- bass kernel
def _ap3(ap, d1, d2):
    return bass.AP(tensor=ap.tensor, offset=ap.offset,
                   ap=[list(ap.ap[0]), list(d1), list(d2)])


def build_nc():
    nc = bacc.Bacc(None, target_bir_lowering=False, debug=False)
    qt_d = nc.declare_dram_parameter("qt", [BH_PER_CORE, KAUG, S], F16,
                                     isOutput=False)
    kpt_d = nc.declare_dram_parameter("kpt", [BH_PER_CORE, KAUG, TOT + 128],
                                      F16, isOutput=False)
    vp_d = nc.declare_dram_parameter("vp", [BH_PER_CORE, 128, NCH * VA], F16,
                                     isOutput=False)
    # out in DMA-friendly permuted layout (2 KiB contiguous runs on both
    # sides); the host unpermutes to [S, D].  Layout per (bh, half):
    # [128 partitions, quarter(2), group(8), d(32)] where group<4 is block
    # 4*qd+group rows 0..128 and group>=4 is block 4*qd+group-4 rows 88..216
    # (the overlap rows 88..128 are shipped twice; host uses rows 40..128).
    out_q_d = nc.declare_dram_parameter(
        "out_q", [BH_PER_CORE, 2, 128, 2, 8, D], F32, isOutput=True)

    dma_eng = {"sync": lambda nc: nc.sync, "gpsimd": lambda nc: nc.gpsimd,
               "vector": lambda nc: nc.vector}[DMA_ENGINE](nc)
    norm_eng = {"vector": lambda nc: nc.vector,
                "gpsimd": lambda nc: nc.gpsimd}[NORM_ENGINE](nc)

    with tile.TileContext(nc) as tc:
        with (
            tc.tile_pool(name="singles", bufs=1) as singles,
            tc.tile_pool(name="qtp", bufs=3) as qtp,
            tc.tile_pool(name="kptp", bufs=3) as kptp,
            tc.tile_pool(name="vpp", bufs=3) as vpp,
            tc.tile_pool(name="probsp", bufs=3) as probsp,
            tc.tile_pool(name="recp", bufs=2) as recp,
            tc.tile_pool(name="outlp", bufs=2) as outlp,
            tc.tile_pool(name="scoresp", bufs=3, space="PSUM") as scoresp,
            tc.tile_pool(name="pvqp", bufs=2, space="PSUM") as pvqp,
        ):
            ebc = singles.tile([128, 1], F32)
            nc.vector.memset(ebc[:], float(np.e))

            for i in range(BH_PER_CORE):
                qt_sb = qtp.tile([KAUG, S], F16)
                kpt_sb = kptp.tile([KAUG, TOT + 128], F16)
                vp_sb = vpp.tile([128, NCH * VA], F16)
                if i == 0:
                    # split first loads so compute can start early
                    dma_eng.dma_start(out=kpt_sb[:, :KHALF],
                                      in_=kpt_d[i, :, :KHALF])
                    dma_eng.dma_start(out=qt_sb[:, :S // 2],
                                      in_=qt_d[i, :, :S // 2])
                    dma_eng.dma_start(out=vp_sb[:], in_=vp_d[i])
                    dma_eng.dma_start(out=kpt_sb[:, KHALF:],
                                      in_=kpt_d[i, :, KHALF:])
                    dma_eng.dma_start(out=qt_sb[:, S // 2:],
                                      in_=qt_d[i, :, S // 2:])
                else:
                    dma_eng.dma_start(out=qt_sb[:], in_=qt_d[i])
                    dma_eng.dma_start(out=kpt_sb[:], in_=kpt_d[i])
                    dma_eng.dma_start(out=vp_sb[:], in_=vp_d[i])

                for half in range(2):
                    out_sb = outlp.tile([128, 512], F32)
                    for pp in range(4):
                        p = 4 * half + pp
                        if pp % 2 == 0:
                            # one PV psum bank per quarter (2 pairs): lo
                            # blocks at cols 33b, hi (q 88..216) at 132+33b
                            pv = pvqp.tile([128, 512], F32)
                            last_pv = None
                        scores = scoresp.tile([128, 1024], F32)
                        probs = probsp.tile([128, 1024], F16)

                        # ---- QK^T: one matmul per chunk (K=34 contraction,
                        # M=128 always); one psum group per bank: start on the
                        # first chunk, stop on the last (all writes are
                        # full-width so eager and lazy hw zeroing agree).
                        qk_stop = []
                        for tb in range(2):
                            t = 2 * p + tb
                            soff = 512 * tb
                            ncs = len(SCHED[t])
                            prev = None
                            for j, c in enumerate(SCHED[t]):
                                mm = nc.tensor.matmul(
                                    scores[0:128, soff:soff + c["cols"]],
                                    lhsT=kpt_sb[:, c["koff"]:c["koff"] + 128],
                                    rhs=qt_sb[:, F * t:F * t + c["cols"]],
                                    start=(j == 0), stop=(j == ncs - 1))
                                if prev is not None:
                                    add_dep_helper(mm.ins, prev.ins, sync=False,
                                                   reason="qk group order")
                                prev = mm
                                soff += c["cols"]
                            qk_stop.append(prev)

                        # ---- probs = exp(scores), split Act / DVE
                        cmax = PAIR_COLS[p]
                        # ragged pair: init the shorter bank's tail columns
                        # so the strided exp APs never read uninitialized psum
                        for tb in range(2):
                            cb = BLK_COLS[2 * p + tb]
                            if cb < cmax:
                                nc.vector.memset(
                                    scores[0:128, 512 * tb + cb:
                                           512 * tb + cmax], 0.0)
                        a = min(ACT_COLS[p], cmax)
                        sc_v = scores[:].rearrange("p (b x) -> p b x", b=2)
                        pr_v = probs[:].rearrange("p (b x) -> p b x", b=2)
                        exp_ops = []
                        if a > 0:
                            exp_ops.append(nc.scalar.activation(
                                pr_v[:, :, 0:a], sc_v[:, :, 0:a],
                                mybir.ActivationFunctionType.Exp))
                        if cmax > a:
                            n = cmax - a
                            e_b = _ap3(ebc[:], (0, 2), (0, n))
                            exp_ops.append(nc.vector.tensor_tensor(
                                _ap3(probs[0:128, a:a + 1], (512, 2), (1, n)),
                                e_b,
                                _ap3(scores[0:128, a:a + 1], (512, 2), (1, n)),
                                op=mybir.AluOpType.pow))
                        # exp may only read a bank once its group is closed
                        for op in exp_ops:
                            for smm in qk_stop:
                                add_dep_helper(op.ins, smm.ins, sync=True,
                                               reason="exp after qk stop")

                        # ---- PV: out[q, 33], accumulate into the quarter's
                        # psum bank.  One accumulation group per bank covers
                        # the whole quarter (start on its very first matmul,
                        # stop on its last); every matmul writes the full 128
                        # partitions except the narrow g2 ones, and every
                        # byte's first writer is full-width, so eager and
                        # lazy hw zeroing agree.  "Upper" covers q 88..216
                        # (128 wide; rows 88..128 duplicate lo, host ignores).
                        for tb in range(2):
                            t = 2 * p + tb
                            bq = 2 * (pp % 2) + tb     # block within quarter
                            is_first = bq == 0
                            is_last = bq == 3
                            mcol = VA * bq
                            ucol = 4 * VA + VA * bq
                            g1c = [c for c in SCHED[t] if c["cols"] == F]
                            g2c = [c for c in SCHED[t] if c["cols"] != F]
                            soff = {id(c): 512 * tb + sum(
                                cc["cols"] for cc in SCHED[t][:j])
                                for j, c in enumerate(SCHED[t])}
                            mms = []
                            for j, c in enumerate([g1c[0]] + g2c + g1c[1:]):
                                w = 128 if c["cols"] == F else IMG_START
                                mms.append(nc.tensor.matmul(
                                    pv[0:w, mcol:mcol + VA],
                                    lhsT=probs[0:c["rows"],
                                               soff[id(c)]:soff[id(c)] + w],
                                    rhs=vp_sb[0:c["rows"],
                                              VA * c["ci"]:VA * c["ci"] + VA],
                                    start=(is_first and j == 0), stop=False))
                            for j, c in enumerate(g1c):
                                mms.append(nc.tensor.matmul(
                                    pv[0:128, ucol:ucol + VA],
                                    lhsT=probs[0:c["rows"],
                                               soff[id(c)] + 88:
                                               soff[id(c)] + 216],
                                    rhs=vp_sb[0:c["rows"],
                                              VA * c["ci"]:VA * c["ci"] + VA],
                                    start=False,
                                    stop=(is_last and j == len(g1c) - 1)))
                            for mm in mms:
                                if last_pv is not None:
                                    add_dep_helper(mm.ins, last_pv.ins,
                                                   sync=False,
                                                   reason="pv group order")
                                last_pv = mm

                        if pp % 2 == 1:
                            # ---- normalize the quarter: out = num * (1/den)
                            qq = pp // 2
                            recips = recp.tile([128, 8], F32)
                            nc.vector.reciprocal(recips[0:128, 0:8],
                                                 pv[0:128, 32:8 * VA:VA])
                            norm_eng.tensor_mul(
                                _ap3(out_sb[0:128, 256 * qq:256 * qq + 1],
                                     (32, 8), (1, 32)),
                                _ap3(pv[0:128, 0:1], (VA, 8), (1, 32)),
                                _ap3(recips[0:128, 0:1], (1, 8), (0, 32)))

                    # ---- store to permuted scratch [bh, half, row, q, g, d]
                    dst = bass.AP(tensor=out_q_d,
                                  offset=(i * 2 + half) * 128 * 512,
                                  ap=[[512, 128], [1, 512]])
                    dma_eng.dma_start(out=dst, in_=out_sb[:])
    nc.compile()
    return nc


_NC = None


def _get_nc():
    global _NC
    if _NC is None:
        _NC = build_nc()
    return _NC


# ---------------------------------------------------------------- entry point
def kernel(q, k, v, feats_per_t, window_len, act_size, img_feat_size):
    assert int(feats_per_t) == F and int(window_len) == W
    assert int(act_size) == 16 and int(img_feat_size) == 196
    q = np.asarray(q, np.float32)
    k = np.asarray(k, np.float32)
    v = np.asarray(v, np.float32)

    qt, kpt, vp = _pack_all(q, k, v)
    in_maps = []
    for core in range(N_CORES):
        s = slice(BH_PER_CORE * core, BH_PER_CORE * (core + 1))
        in_maps.append({"qt": np.ascontiguousarray(qt[s]),
                        "kpt": np.ascontiguousarray(kpt[s]),
                        "vp": np.ascontiguousarray(vp[s])})

    nc = _get_nc()
    res = run_bass_kernel_spmd(nc, in_maps, list(range(N_CORES)))
    out = np.empty((B * H, 2, 2, 4, F, D), np.float32)
    for core in range(N_CORES):
        s = slice(BH_PER_CORE * core, BH_PER_CORE * (core + 1))
        sc = res.results[core]["out_q"]    # [3, 2, 128, 2, 8, D]
        out[s, :, :, :, :128] = sc[:, :, :, :, 0:4].transpose(0, 1, 3, 4, 2, 5)
        out[s, :, :, :, 128:] = sc[:, :, 40:, :, 4:8].transpose(0, 1, 3, 4, 2, 5)
    return np.ascontiguousarray(out.reshape(B, H, S, D))


# revision 23
# speedup vs baseline: 1.0010x; 1.0010x over previous
"""Trainium2 Bass kernel for nn_EyeRobotAgent block-sparse ("eye") attention.

Shapes: q,k,v [2, 12, 3456, 32] fp32.  S = 16 time-blocks x 216 feats.
Visibility per query block t:
  - g1 keys (visible to ALL 216 q of the block): the 20 non-img keys of
    block t, plus 19 keys (m in {0..3, 5..19}) of each past block
    t-7..t-1 (joint q additionally can't see past joint keys -> folded
    into the QK matmul as a bias row),
  - g2 keys (visible ONLY to the 20 non-img q): the 196 img keys of
    block t (img q never see img keys -> bias row for mixed chunks).

Strategy (data-parallel: 24 (b,h) pairs over 8 cores, 3 each):
  Per block, pack [g1 | g2] key rows into 128-partition chunks; chunks
  containing any g1 row stream all 216 q columns, pure-g2 chunks stream
  only the 20 non-img q columns.  Early blocks have fewer past keys and
  fewer/smaller chunks.  Scores are computed transposed [kv, q] with the
  two mask predicates folded in as 2 extra contraction rows; exp runs
  split between the Act engine (Exp) and the DVE (pow(e, x)) so neither
  is a serial bottleneck; probs feed PV matmuls oriented out[q, 33]
  (33-column streams; column 32 is the softmax denominator via a ones
  column in V).  PV accumulates 8 blocks (a half-(b,h)) per PSUM bank,
  then one batched reciprocal+multiply normalizes and stores.
"""
import numpy as np

import concourse.bass as bass
import concourse.mybir as mybir
import concourse.tile as tile
from concourse import bacc
from concourse.bass_utils import run_bass_kernel_spmd
from concourse.tile_rust import add_dep_helper

B, H, S, D = 2, 12, 3456, 32
F = 216            # feats_per_t
W = 8              # window_len
T = S // F         # 16 blocks
IMG_START = 20     # F - img_feat_size
JOINT_START = 4    # IMG_START - act_size
PAST_SEL = np.array([0, 1, 2, 3] + list(range(5, 20)))   # 19 per past block
NEG = np.float32(-30000.0)
SCALE = float(1.0 / np.sqrt(np.float32(D)))
N_CORES = 8
BH_PER_CORE = (B * H) // N_CORES      # 3
KAUG = D + 2       # 32 d + img-bias row + joint-bias row
VA = D + 1         # 33 = v columns + ones column

F32 = mybir.dt.float32
F16 = mybir.dt.float16
NP_F16 = np.float16


# ------------------------------------------------------------- chunk schedule
def _schedule():
    """Per block t: list of chunks dict(rows, cols, keys, koff, ci)."""
    scheds = []
    koff = 0
    ci = 0
    for t in range(T):
        g1 = [t * F + m for m in range(IMG_START)]
        for tau in range(max(0, t - (W - 1)), t):
            g1 += [tau * F + int(m) for m in PAST_SEL]
        g2 = [t * F + m for m in range(IMG_START, F)]
        keys = np.array(g1 + g2)
        n1 = len(g1)
        chunks = []
        off = 0
        while off < len(keys):
            n = int(min(128, len(keys) - off))
            cols = F if off < n1 else IMG_START
            chunks.append(dict(rows=n, cols=cols, keys=keys[off:off + n],
                               koff=koff, ci=ci))
            koff += n
            ci += 1
            off += n
        scheds.append(chunks)
    return scheds, koff, ci


SCHED, TOT, NCH = _schedule()
BLK_COLS = [sum(c["cols"] for c in SCHED[t]) for t in range(T)]
PAIR_COLS = [max(BLK_COLS[2 * p], BLK_COLS[2 * p + 1]) for p in range(T // 2)]
KHALF = sum(c["rows"] for t in range(T // 2) for c in SCHED[t])
CHALF = sum(len(SCHED[t]) for t in range(T // 2))

# exp engine split per pair: Act handles cols [0, a), DVE handles [a, cmax)
# (DVE is ~1.25x slower per col and also runs the reciprocals; Act pays a
# bigger fixed per-op cost).  Tuned against the CoreSim cost model.
ACT_COLS = [170, 184, 184, 325, 325, 325, 325, 325]

NORM_ENGINE = "vector"     # engine for the normalize multiplies
DMA_ENGINE = "sync"        # SP-issued DMAs run async on the HWDGE queues


# ---------------------------------------------------------------- host packing
def _pack_all(q, k, v):
    """q,k,v: [B,H,S,D] fp32 ->
       qt  [24, KAUG, S]    (augmented scaled Q^T)
       kpt [24, KAUG, TOT]  (augmented packed K^T, chunk-major)
       vp  [24, 128, NCH*VA] (packed V + ones column per chunk)"""
    nbh = B * H
    qf = q.reshape(nbh, S, D)
    kf = k.reshape(nbh, S, D)
    vf = v.reshape(nbh, S, D)

    qm = np.arange(S) % F
    qt = np.zeros((nbh, KAUG, S), np.float32)
    qt[:, :D] = (qf * SCALE).transpose(0, 2, 1)
    qt[:, 32] = qm >= IMG_START
    qt[:, 33] = (qm >= JOINT_START) & (qm < IMG_START)

    keys_all = np.concatenate([c["keys"] for t in range(T) for c in SCHED[t]])
    km = keys_all % F
    kt = keys_all // F
    # block id each key row belongs to (for past-joint bias)
    owner = np.concatenate([np.full(c["rows"], t)
                            for t in range(T) for c in SCHED[t]])
    kpt = np.zeros((nbh, KAUG, TOT + 128), np.float32)
    kpt[:, :D, :TOT] = kf[:, keys_all].transpose(0, 2, 1)
    kpt[:, 32, :TOT] = np.where(km >= IMG_START, NEG, 0.0)
    kpt[:, 33, :TOT] = np.where((km >= JOINT_START) & (km < IMG_START)
                                & (kt < owner), NEG, 0.0)

    vp = np.zeros((nbh, NCH, 128, VA), np.float32)
    for t in range(T):
        for c in SCHED[t]:
            r = c["rows"]
            vp[:, c["ci"], :r, :D] = vf[:, c["keys"]]
            vp[:, c["ci"], :r, 32] = 1.0
    vp = np.ascontiguousarray(
        vp.transpose(0, 2, 1, 3).reshape(nbh, 128, NCH * VA))
    return (qt.astype(NP_F16), kpt.astype(NP_F16), vp.astype(NP_F16))


# ---------------------------------------------------------------- bass kernel
def _ap3(ap, d1, d2):
    return bass.AP(tensor=ap.tensor, offset=ap.offset,
                   ap=[list(ap.ap[0]), list(d1), list(d2)])


def build_nc():
    nc = bacc.Bacc(None, target_bir_lowering=False, debug=False)
    qt_d = nc.declare_dram_parameter("qt", [BH_PER_CORE, KAUG, S], F16,
                                     isOutput=False)
    kpt_d = nc.declare_dram_parameter("kpt", [BH_PER_CORE, KAUG, TOT + 128],
                                      F16, isOutput=False)
    vp_d = nc.declare_dram_parameter("vp", [BH_PER_CORE, 128, NCH * VA], F16,
                                     isOutput=False)
    # out in DMA-friendly permuted layout (2 KiB contiguous runs on both
    # sides); the host unpermutes to [S, D].  Layout per (bh, half):
    # [128 partitions, quarter(2), group(8), d(32)] where group<4 is block
    # 4*qd+group rows 0..128 and group>=4 is block 4*qd+group-4 rows 88..216
    # (the overlap rows 88..128 are shipped twice; host uses rows 40..128).
    out_q_d = nc.declare_dram_parameter(
        "out_q", [BH_PER_CORE, 2, 128, 2, 8, D], F32, isOutput=True)

    dma_eng = {"sync": lambda nc: nc.sync, "gpsimd": lambda nc: nc.gpsimd,
               "vector": lambda nc: nc.vector}[DMA_ENGINE](nc)
    norm_eng = {"vector": lambda nc: nc.vector,
                "gpsimd": lambda nc: nc.gpsimd}[NORM_ENGINE](nc)

    with tile.TileContext(nc) as tc:
        with (
            tc.tile_pool(name="singles", bufs=1) as singles,
            tc.tile_pool(name="qtp", bufs=3) as qtp,
            tc.tile_pool(name="kptp", bufs=3) as kptp,
            tc.tile_pool(name="vpp", bufs=3) as vpp,
            tc.tile_pool(name="probsp", bufs=3) as probsp,
            tc.tile_pool(name="recp", bufs=2) as recp,
            tc.tile_pool(name="outlp", bufs=2) as outlp,
            tc.tile_pool(name="scoresp", bufs=3, space="PSUM") as scoresp,
            tc.tile_pool(name="pvqp", bufs=2, space="PSUM") as pvqp,
        ):
            ebc = singles.tile([128, 1], F32)
            nc.vector.memset(ebc[:], float(np.e))

            # hoist ALL input DMAs to the program start: the SP queue is
            # in-order, so an out-store waiting on compute must never sit
            # ahead of a later bh's input loads
            tiles = []
            for i in range(BH_PER_CORE):
                qt_sb = qtp.tile([KAUG, S], F16)
                kpt_sb = kptp.tile([KAUG, TOT + 128], F16)
                vp_sb = vpp.tile([128, NCH * VA], F16)
                if i == 0:
                    # split first loads so compute can start early
                    dma_eng.dma_start(out=kpt_sb[:, :KHALF],
                                      in_=kpt_d[i, :, :KHALF])
                    dma_eng.dma_start(out=qt_sb[:, :S // 2],
                                      in_=qt_d[i, :, :S // 2])
                    dma_eng.dma_start(out=vp_sb[:], in_=vp_d[i])
                    dma_eng.dma_start(out=kpt_sb[:, KHALF:],
                                      in_=kpt_d[i, :, KHALF:])
                    dma_eng.dma_start(out=qt_sb[:, S // 2:],
                                      in_=qt_d[i, :, S // 2:])
                else:
                    dma_eng.dma_start(out=qt_sb[:], in_=qt_d[i])
                    dma_eng.dma_start(out=kpt_sb[:], in_=kpt_d[i])
                    dma_eng.dma_start(out=vp_sb[:], in_=vp_d[i])
                tiles.append((qt_sb, kpt_sb, vp_sb))

            for i in range(BH_PER_CORE):
                qt_sb, kpt_sb, vp_sb = tiles[i]

                for half in range(2):
                    out_sb = outlp.tile([128, 512], F32)
                    for pp in range(4):
                        p = 4 * half + pp
                        if pp % 2 == 0:
                            # one PV psum bank per quarter (2 pairs): lo
                            # blocks at cols 33b, hi (q 88..216) at 132+33b
                            pv = pvqp.tile([128, 512], F32)
                            last_pv = None
                        scores = scoresp.tile([128, 1024], F32)
                        probs = probsp.tile([128, 1024], F16)

                        # ---- QK^T: one matmul per chunk (K=34 contraction,
                        # M=128 always); one psum group per bank: start on the
                        # first chunk, stop on the last (all writes are
                        # full-width so eager and lazy hw zeroing agree).
                        qk_stop = []
                        for tb in range(2):
                            t = 2 * p + tb
                            soff = 512 * tb
                            ncs = len(SCHED[t])
                            prev = None
                            for j, c in enumerate(SCHED[t]):
                                mm = nc.tensor.matmul(
                                    scores[0:128, soff:soff + c["cols"]],
                                    lhsT=kpt_sb[:, c["koff"]:c["koff"] + 128],
                                    rhs=qt_sb[:, F * t:F * t + c["cols"]],
                                    start=(j == 0), stop=(j == ncs - 1))
                                if prev is not None:
                                    add_dep_helper(mm.ins, prev.ins, sync=False,
                                                   reason="qk group order")
                                prev = mm
                                soff += c["cols"]
                            qk_stop.append(prev)

                        # ---- probs = exp(scores), split Act / DVE
                        cmax = PAIR_COLS[p]
                        # ragged pair: init the shorter bank's tail columns
                        # so the strided exp APs never read uninitialized psum
                        for tb in range(2):
                            cb = BLK_COLS[2 * p + tb]
                            if cb < cmax:
                                nc.vector.memset(
                                    scores[0:128, 512 * tb + cb:
                                           512 * tb + cmax], 0.0)
                        a = min(ACT_COLS[p], cmax)
                        sc_v = scores[:].rearrange("p (b x) -> p b x", b=2)
                        pr_v = probs[:].rearrange("p (b x) -> p b x", b=2)
                        exp_ops = []
                        if a > 0:
                            exp_ops.append(nc.scalar.activation(
                                pr_v[:, :, 0:a], sc_v[:, :, 0:a],
                                mybir.ActivationFunctionType.Exp))
                        if cmax > a:
                            n = cmax - a
                            e_b = _ap3(ebc[:], (0, 2), (0, n))
                            exp_ops.append(nc.vector.tensor_tensor(
                                _ap3(probs[0:128, a:a + 1], (512, 2), (1, n)),
                                e_b,
                                _ap3(scores[0:128, a:a + 1], (512, 2), (1, n)),
                                op=mybir.AluOpType.pow))
                        # exp may only read a bank once its group is closed
                        for op in exp_ops:
                            for smm in qk_stop:
                                add_dep_helper(op.ins, smm.ins, sync=True,
                                               reason="exp after qk stop")

                        # ---- PV: out[q, 33], accumulate into the quarter's
                        # psum bank.  One accumulation group per bank covers
                        # the whole quarter (start on its very first matmul,
                        # stop on its last); every matmul writes the full 128
                        # partitions except the narrow g2 ones, and every
                        # byte's first writer is full-width, so eager and
                        # lazy hw zeroing agree.  "Upper" covers q 88..216
                        # (128 wide; rows 88..128 duplicate lo, host ignores).
                        for tb in range(2):
                            t = 2 * p + tb
                            bq = 2 * (pp % 2) + tb     # block within quarter
                            is_first = bq == 0
                            is_last = bq == 3
                            mcol = VA * bq
                            ucol = 4 * VA + VA * bq
                            g1c = [c for c in SCHED[t] if c["cols"] == F]
                            g2c = [c for c in SCHED[t] if c["cols"] != F]
                            soff = {id(c): 512 * tb + sum(
                                cc["cols"] for cc in SCHED[t][:j])
                                for j, c in enumerate(SCHED[t])}
                            mms = []
                            for j, c in enumerate([g1c[0]] + g2c + g1c[1:]):
                                w = 128 if c["cols"] == F else IMG_START
                                mms.append(nc.tensor.matmul(
                                    pv[0:w, mcol:mcol + VA],
                                    lhsT=probs[0:c["rows"],
                                               soff[id(c)]:soff[id(c)] + w],
                                    rhs=vp_sb[0:c["rows"],
                                              VA * c["ci"]:VA * c["ci"] + VA],
                                    start=(is_first and j == 0), stop=False))
                            for j, c in enumerate(g1c):
                                mms.append(nc.tensor.matmul(
                                    pv[0:128, ucol:ucol + VA],
                                    lhsT=probs[0:c["rows"],
                                               soff[id(c)] + 88:
                                               soff[id(c)] + 216],
                                    rhs=vp_sb[0:c["rows"],
                                              VA * c["ci"]:VA * c["ci"] + VA],
                                    start=False,
                                    stop=(is_last and j == len(g1c) - 1)))
                            for mm in mms:
                                if last_pv is not None:
                                    add_dep_helper(mm.ins, last_pv.ins,
                                                   sync=False,
                                                   reason="pv group order")
                                last_pv = mm

                        if pp % 2 == 1:
                            # ---- normalize the quarter: out = num * (1/den)
                            qq = pp // 2
                            recips = recp.tile([128, 8], F32)
                            nc.vector.reciprocal(recips[0:128, 0:8],
                                                 pv[0:128, 32:8 * VA:VA])
                            norm_eng.tensor_mul(
                                _ap3(out_sb[0:128, 256 * qq:256 * qq + 1],
                                     (32, 8), (1, 32)),
                                _ap3(pv[0:128, 0:1], (VA, 8), (1, 32)),
                                _ap3(recips[0:128, 0:1], (1, 8), (0, 32)))

                    # ---- store to permuted scratch [bh, half, row, q, g, d]
                    dst = bass.AP(tensor=out_q_d,
                                  offset=(i * 2 + half) * 128 * 512,
                                  ap=[[512, 128], [1, 512]])
                    dma_eng.dma_start(out=dst, in_=out_sb[:])
    nc.compile()
    return nc


_NC = None


def _get_nc():
    global _NC
    if _NC is None:
        _NC = build_nc()
    return _NC


# ---------------------------------------------------------------- entry point
def kernel(q, k, v, feats_per_t, window_len, act_size, img_feat_size):
    assert int(feats_per_t) == F and int(window_len) == W
    assert int(act_size) == 16 and int(img_feat_size) == 196
    q = np.asarray(q, np.float32)
    k = np.asarray(k, np.float32)
    v = np.asarray(v, np.float32)

    qt, kpt, vp = _pack_all(q, k, v)
    in_maps = []
    for core in range(N_CORES):
        s = slice(BH_PER_CORE * core, BH_PER_CORE * (core + 1))
        in_maps.append({"qt": np.ascontiguousarray(qt[s]),
                        "kpt": np.ascontiguousarray(kpt[s]),
                        "vp": np.ascontiguousarray(vp[s])})

    nc = _get_nc()
    res = run_bass_kernel_spmd(nc, in_maps, list(range(N_CORES)))
    out = np.empty((B * H, 2, 2, 4, F, D), np.float32)
    for core in range(N_CORES):
        s = slice(BH_PER_CORE * core, BH_PER_CORE * (core + 1))
        sc = res.results[core]["out_q"]    # [3, 2, 128, 2, 8, D]
        out[s, :, :, :, :128] = sc[:, :, :, :, 0:4].transpose(0, 1, 3, 4, 2, 5)
        out[s, :, :, :, 128:] = sc[:, :, 40:, :, 4:8].transpose(0, 1, 3, 4, 2, 5)
    return np.ascontiguousarray(out.reshape(B, H, S, D))


# revision 24
# speedup vs baseline: 1.2593x; 1.2581x over previous
"""Trainium2 Bass kernel for nn_EyeRobotAgent block-sparse ("eye") attention.

Shapes: q,k,v [2, 12, 3456, 32] fp32.  S = 16 time-blocks x 216 feats.
Visibility per query block t:
  - g1 keys (visible to ALL 216 q of the block): the 20 non-img keys of
    block t, plus 19 keys (m in {0..3, 5..19}) of each past block
    t-7..t-1 (joint q additionally can't see past joint keys -> folded
    into the QK matmul as a bias row),
  - g2 keys (visible ONLY to the 20 non-img q): the 196 img keys of
    block t (img q never see img keys -> bias row for mixed chunks).

Strategy (data-parallel: 24 (b,h) pairs over 8 cores, 3 each):
  Per block, pack [g1 | g2] key rows into 128-partition chunks; chunks
  containing any g1 row stream all 216 q columns, pure-g2 chunks stream
  only the 20 non-img q columns.  Early blocks have fewer past keys and
  fewer/smaller chunks.  Scores are computed transposed [kv, q] with the
  two mask predicates folded in as 2 extra contraction rows; exp runs
  split between the Act engine (Exp) and the DVE (pow(e, x)) so neither
  is a serial bottleneck; probs feed PV matmuls oriented out[q, 33]
  (33-column streams; column 32 is the softmax denominator via a ones
  column in V).  PV accumulates 8 blocks (a half-(b,h)) per PSUM bank,
  then one batched reciprocal+multiply normalizes and stores.
"""
import numpy as np

import concourse.bass as bass
import concourse.mybir as mybir
import concourse.tile as tile
from concourse import bacc
from concourse.bass_utils import run_bass_kernel_spmd
from concourse.tile_rust import add_dep_helper

B, H, S, D = 2, 12, 3456, 32
F = 216            # feats_per_t
W = 8              # window_len
T = S // F         # 16 blocks
IMG_START = 20     # F - img_feat_size
JOINT_START = 4    # IMG_START - act_size
PAST_SEL = np.array([0, 1, 2, 3] + list(range(5, 20)))   # 19 per past block
NEG = np.float32(-30000.0)
SCALE = float(1.0 / np.sqrt(np.float32(D)))
N_CORES = 8
BH_PER_CORE = (B * H) // N_CORES      # 3
KAUG = D + 2       # 32 d + img-bias row + joint-bias row
VA = D + 1         # 33 = v columns + ones column

F32 = mybir.dt.float32
F16 = mybir.dt.float16
NP_F16 = np.float16


# ------------------------------------------------------------- chunk schedule
def _schedule():
    """Per block t: list of chunks dict(rows, cols, keys, koff, ci)."""
    scheds = []
    koff = 0
    ci = 0
    for t in range(T):
        g1 = [t * F + m for m in range(IMG_START)]
        for tau in range(max(0, t - (W - 1)), t):
            g1 += [tau * F + int(m) for m in PAST_SEL]
        g2 = [t * F + m for m in range(IMG_START, F)]
        keys = np.array(g1 + g2)
        n1 = len(g1)
        chunks = []
        off = 0
        while off < len(keys):
            n = int(min(128, len(keys) - off))
            cols = F if off < n1 else IMG_START
            chunks.append(dict(rows=n, cols=cols, keys=keys[off:off + n],
                               koff=koff, ci=ci))
            koff += n
            ci += 1
            off += n
        scheds.append(chunks)
    return scheds, koff, ci


SCHED, TOT, NCH = _schedule()
BLK_COLS = [sum(c["cols"] for c in SCHED[t]) for t in range(T)]
PAIR_COLS = [max(BLK_COLS[2 * p], BLK_COLS[2 * p + 1]) for p in range(T // 2)]
KHALF = sum(c["rows"] for t in range(T // 2) for c in SCHED[t])
CHALF = sum(len(SCHED[t]) for t in range(T // 2))

# exp engine split per pair: Act handles cols [0, a), DVE handles [a, cmax)
# (DVE is ~1.25x slower per col and also runs the reciprocals; Act pays a
# bigger fixed per-op cost).  Tuned against the CoreSim cost model.
ACT_COLS = [170, 184, 184, 325, 325, 325, 325, 325]

NORM_ENGINE = "vector"     # engine for the normalize multiplies
DMA_ENGINE = "sync"        # SP-issued DMAs run async on the HWDGE queues


# ---------------------------------------------------------------- host packing
def _pack_all(q, k, v):
    """q,k,v: [B,H,S,D] fp32 ->
       qt  [24, KAUG, S]    (augmented scaled Q^T)
       kpt [24, KAUG, TOT]  (augmented packed K^T, chunk-major)
       vp  [24, 128, NCH*VA] (packed V + ones column per chunk)"""
    nbh = B * H
    qf = q.reshape(nbh, S, D)
    kf = k.reshape(nbh, S, D)
    vf = v.reshape(nbh, S, D)

    qm = np.arange(S) % F
    qt = np.zeros((nbh, KAUG, S), np.float32)
    qt[:, :D] = (qf * SCALE).transpose(0, 2, 1)
    qt[:, 32] = qm >= IMG_START
    qt[:, 33] = (qm >= JOINT_START) & (qm < IMG_START)

    keys_all = np.concatenate([c["keys"] for t in range(T) for c in SCHED[t]])
    km = keys_all % F
    kt = keys_all // F
    # block id each key row belongs to (for past-joint bias)
    owner = np.concatenate([np.full(c["rows"], t)
                            for t in range(T) for c in SCHED[t]])
    kpt = np.zeros((nbh, KAUG, TOT + 128), np.float32)
    kpt[:, :D, :TOT] = kf[:, keys_all].transpose(0, 2, 1)
    kpt[:, 32, :TOT] = np.where(km >= IMG_START, NEG, 0.0)
    kpt[:, 33, :TOT] = np.where((km >= JOINT_START) & (km < IMG_START)
                                & (kt < owner), NEG, 0.0)

    vp = np.zeros((nbh, NCH, 128, VA), np.float32)
    for t in range(T):
        for c in SCHED[t]:
            r = c["rows"]
            vp[:, c["ci"], :r, :D] = vf[:, c["keys"]]
            vp[:, c["ci"], :r, 32] = 1.0
    vp = np.ascontiguousarray(
        vp.transpose(0, 2, 1, 3).reshape(nbh, 128, NCH * VA))
    return (qt.astype(NP_F16), kpt.astype(NP_F16), vp.astype(NP_F16))


# ---------------------------------------------------------------- bass kernel
def _ap3(ap, d1, d2):
    return bass.AP(tensor=ap.tensor, offset=ap.offset,
                   ap=[list(ap.ap[0]), list(d1), list(d2)])


def build_nc():
    nc = bacc.Bacc(None, target_bir_lowering=False, debug=False)
    qt_d = nc.declare_dram_parameter("qt", [BH_PER_CORE, KAUG, S], F16,
                                     isOutput=False)
    kpt_d = nc.declare_dram_parameter("kpt", [BH_PER_CORE, KAUG, TOT + 128],
                                      F16, isOutput=False)
    vp_d = nc.declare_dram_parameter("vp", [BH_PER_CORE, 128, NCH * VA], F16,
                                     isOutput=False)
    # out in DMA-friendly permuted layout (2 KiB contiguous runs on both
    # sides); the host unpermutes to [S, D].  Layout per (bh, half):
    # [128 partitions, quarter(2), group(8), d(32)] where group<4 is block
    # 4*qd+group rows 0..128 and group>=4 is block 4*qd+group-4 rows 88..216
    # (the overlap rows 88..128 are shipped twice; host uses rows 40..128).
    out_q_d = nc.declare_dram_parameter(
        "out_q", [BH_PER_CORE, 2, 128, 2, 8, D], F32, isOutput=True)

    dma_eng = {"sync": lambda nc: nc.sync, "gpsimd": lambda nc: nc.gpsimd,
               "vector": lambda nc: nc.vector}[DMA_ENGINE](nc)
    norm_eng = {"vector": lambda nc: nc.vector,
                "gpsimd": lambda nc: nc.gpsimd}[NORM_ENGINE](nc)

    with tile.TileContext(nc) as tc:
        with (
            tc.tile_pool(name="singles", bufs=1) as singles,
            tc.tile_pool(name="qtp", bufs=3) as qtp,
            tc.tile_pool(name="kptp", bufs=3) as kptp,
            tc.tile_pool(name="vpp", bufs=3) as vpp,
            tc.tile_pool(name="probsp", bufs=3) as probsp,
            tc.tile_pool(name="recp", bufs=2) as recp,
            tc.tile_pool(name="outlp", bufs=2) as outlp,
            tc.tile_pool(name="scoresp", bufs=3, space="PSUM") as scoresp,
            tc.tile_pool(name="pvqp", bufs=2, space="PSUM") as pvqp,
        ):
            ebc = singles.tile([128, 1], F32)
            nc.vector.memset(ebc[:], float(np.e))

            # hoist ALL input DMAs to the program start: the SP queue is
            # in-order, so an out-store waiting on compute must never sit
            # ahead of a later bh's input loads
            tiles = []
            for i in range(BH_PER_CORE):
                qt_sb = qtp.tile([KAUG, S], F16)
                kpt_sb = kptp.tile([KAUG, TOT + 128], F16)
                vp_sb = vpp.tile([128, NCH * VA], F16)
                if i == 0:
                    # split first loads so compute can start early
                    dma_eng.dma_start(out=kpt_sb[:, :KHALF],
                                      in_=kpt_d[i, :, :KHALF])
                    dma_eng.dma_start(out=qt_sb[:, :S // 2],
                                      in_=qt_d[i, :, :S // 2])
                    dma_eng.dma_start(out=vp_sb[:], in_=vp_d[i])
                    dma_eng.dma_start(out=kpt_sb[:, KHALF:],
                                      in_=kpt_d[i, :, KHALF:])
                    dma_eng.dma_start(out=qt_sb[:, S // 2:],
                                      in_=qt_d[i, :, S // 2:])
                else:
                    # later bh loads ride the (otherwise idle) Pool engine's
                    # software-DGE chain, in parallel with SP's chain
                    nc.gpsimd.dma_start(out=qt_sb[:], in_=qt_d[i])
                    nc.gpsimd.dma_start(out=kpt_sb[:], in_=kpt_d[i])
                    nc.gpsimd.dma_start(out=vp_sb[:], in_=vp_d[i])
                tiles.append((qt_sb, kpt_sb, vp_sb))

            for i in range(BH_PER_CORE):
                qt_sb, kpt_sb, vp_sb = tiles[i]

                for half in range(2):
                    out_sb = outlp.tile([128, 512], F32)
                    for pp in range(4):
                        p = 4 * half + pp
                        if pp % 2 == 0:
                            # one PV psum bank per quarter (2 pairs): lo
                            # blocks at cols 33b, hi (q 88..216) at 132+33b
                            pv = pvqp.tile([128, 512], F32)
                            last_pv = None
                        scores = scoresp.tile([128, 1024], F32)
                        probs = probsp.tile([128, 1024], F16)

                        # ---- QK^T: one matmul per chunk (K=34 contraction,
                        # M=128 always); one psum group per bank: start on the
                        # first chunk, stop on the last (all writes are
                        # full-width so eager and lazy hw zeroing agree).
                        qk_stop = []
                        for tb in range(2):
                            t = 2 * p + tb
                            soff = 512 * tb
                            ncs = len(SCHED[t])
                            prev = None
                            for j, c in enumerate(SCHED[t]):
                                mm = nc.tensor.matmul(
                                    scores[0:128, soff:soff + c["cols"]],
                                    lhsT=kpt_sb[:, c["koff"]:c["koff"] + 128],
                                    rhs=qt_sb[:, F * t:F * t + c["cols"]],
                                    start=(j == 0), stop=(j == ncs - 1))
                                if prev is not None:
                                    add_dep_helper(mm.ins, prev.ins, sync=False,
                                                   reason="qk group order")
                                prev = mm
                                soff += c["cols"]
                            qk_stop.append(prev)

                        # ---- probs = exp(scores), split Act / DVE
                        cmax = PAIR_COLS[p]
                        # ragged pair: init the shorter bank's tail columns
                        # so the strided exp APs never read uninitialized psum
                        for tb in range(2):
                            cb = BLK_COLS[2 * p + tb]
                            if cb < cmax:
                                nc.vector.memset(
                                    scores[0:128, 512 * tb + cb:
                                           512 * tb + cmax], 0.0)
                        a = min(ACT_COLS[p], cmax)
                        sc_v = scores[:].rearrange("p (b x) -> p b x", b=2)
                        pr_v = probs[:].rearrange("p (b x) -> p b x", b=2)
                        exp_ops = []
                        if a > 0:
                            exp_ops.append(nc.scalar.activation(
                                pr_v[:, :, 0:a], sc_v[:, :, 0:a],
                                mybir.ActivationFunctionType.Exp))
                        if cmax > a:
                            n = cmax - a
                            e_b = _ap3(ebc[:], (0, 2), (0, n))
                            exp_ops.append(nc.vector.tensor_tensor(
                                _ap3(probs[0:128, a:a + 1], (512, 2), (1, n)),
                                e_b,
                                _ap3(scores[0:128, a:a + 1], (512, 2), (1, n)),
                                op=mybir.AluOpType.pow))
                        # exp may only read a bank once its group is closed
                        for op in exp_ops:
                            for smm in qk_stop:
                                add_dep_helper(op.ins, smm.ins, sync=True,
                                               reason="exp after qk stop")

                        # ---- PV: out[q, 33], accumulate into the quarter's
                        # psum bank.  One accumulation group per bank covers
                        # the whole quarter (start on its very first matmul,
                        # stop on its last); every matmul writes the full 128
                        # partitions except the narrow g2 ones, and every
                        # byte's first writer is full-width, so eager and
                        # lazy hw zeroing agree.  "Upper" covers q 88..216
                        # (128 wide; rows 88..128 duplicate lo, host ignores).
                        for tb in range(2):
                            t = 2 * p + tb
                            bq = 2 * (pp % 2) + tb     # block within quarter
                            is_first = bq == 0
                            is_last = bq == 3
                            mcol = VA * bq
                            ucol = 4 * VA + VA * bq
                            g1c = [c for c in SCHED[t] if c["cols"] == F]
                            g2c = [c for c in SCHED[t] if c["cols"] != F]
                            soff = {id(c): 512 * tb + sum(
                                cc["cols"] for cc in SCHED[t][:j])
                                for j, c in enumerate(SCHED[t])}
                            mms = []
                            for j, c in enumerate([g1c[0]] + g2c + g1c[1:]):
                                w = 128 if c["cols"] == F else IMG_START
                                mms.append(nc.tensor.matmul(
                                    pv[0:w, mcol:mcol + VA],
                                    lhsT=probs[0:c["rows"],
                                               soff[id(c)]:soff[id(c)] + w],
                                    rhs=vp_sb[0:c["rows"],
                                              VA * c["ci"]:VA * c["ci"] + VA],
                                    start=(is_first and j == 0), stop=False))
                            for j, c in enumerate(g1c):
                                mms.append(nc.tensor.matmul(
                                    pv[0:128, ucol:ucol + VA],
                                    lhsT=probs[0:c["rows"],
                                               soff[id(c)] + 88:
                                               soff[id(c)] + 216],
                                    rhs=vp_sb[0:c["rows"],
                                              VA * c["ci"]:VA * c["ci"] + VA],
                                    start=False,
                                    stop=(is_last and j == len(g1c) - 1)))
                            for mm in mms:
                                if last_pv is not None:
                                    add_dep_helper(mm.ins, last_pv.ins,
                                                   sync=False,
                                                   reason="pv group order")
                                last_pv = mm

                        if pp % 2 == 1:
                            # ---- normalize the quarter: out = num * (1/den)
                            qq = pp // 2
                            recips = recp.tile([128, 8], F32)
                            nc.vector.reciprocal(recips[0:128, 0:8],
                                                 pv[0:128, 32:8 * VA:VA])
                            norm_eng.tensor_mul(
                                _ap3(out_sb[0:128, 256 * qq:256 * qq + 1],
                                     (32, 8), (1, 32)),
                                _ap3(pv[0:128, 0:1], (VA, 8), (1, 32)),
                                _ap3(recips[0:128, 0:1], (1, 8), (0, 32)))

                    # ---- store to permuted scratch [bh, half, row, q, g, d]
                    dst = bass.AP(tensor=out_q_d,
                                  offset=(i * 2 + half) * 128 * 512,
                                  ap=[[512, 128], [1, 512]])
                    dma_eng.dma_start(out=dst, in_=out_sb[:])
    nc.compile()
    return nc


_NC = None


def _get_nc():
    global _NC
    if _NC is None:
        _NC = build_nc()
    return _NC


# ---------------------------------------------------------------- entry point
def kernel(q, k, v, feats_per_t, window_len, act_size, img_feat_size):
    assert int(feats_per_t) == F and int(window_len) == W
    assert int(act_size) == 16 and int(img_feat_size) == 196
    q = np.asarray(q, np.float32)
    k = np.asarray(k, np.float32)
    v = np.asarray(v, np.float32)

    qt, kpt, vp = _pack_all(q, k, v)
    in_maps = []
    for core in range(N_CORES):
        s = slice(BH_PER_CORE * core, BH_PER_CORE * (core + 1))
        in_maps.append({"qt": np.ascontiguousarray(qt[s]),
                        "kpt": np.ascontiguousarray(kpt[s]),
                        "vp": np.ascontiguousarray(vp[s])})

    nc = _get_nc()
    res = run_bass_kernel_spmd(nc, in_maps, list(range(N_CORES)))
    out = np.empty((B * H, 2, 2, 4, F, D), np.float32)
    for core in range(N_CORES):
        s = slice(BH_PER_CORE * core, BH_PER_CORE * (core + 1))
        sc = res.results[core]["out_q"]    # [3, 2, 128, 2, 8, D]
        out[s, :, :, :, :128] = sc[:, :, :, :, 0:4].transpose(0, 1, 3, 4, 2, 5)
        out[s, :, :, :, 128:] = sc[:, :, 40:, :, 4:8].transpose(0, 1, 3, 4, 2, 5)
    return np.ascontiguousarray(out.reshape(B, H, S, D))
